# revision 1
# baseline (speedup 1.0000x reference)
"""MultiHeadLinearAttention Trainium2 kernel (8-core SPMD, fp8 DoubleRow GLU).

Sharding: 16384 tokens split across 8 cores (core c: batch c//2, sequence half
c%2). All projections/attention/out-proj are local; the only cross-core
dependency is the per-batch KV summary (kv [H,DK,DK] + ksum [D]) reduced via a
266KB pair-wise AllReduce, overlapped with early stage-2 compute.

Matmul scheme (the six GLU matmuls, ~86% of PE work):
  - host splits x and 16*W each into two fp8e4m3 levels (hi = fp8(a),
    lo = fp8(a - hi)); the x16 weight pre-scaling keeps the lo level inside
    e4m3's normal range (raw residuals of U(+-1/32) weights sit below the
    2^-9 denormal floor). The 1/16 is folded back via ACT's scale param.
  - each 1024-deep contraction runs as 3 DoubleRow streams (xh*wh + xl*wh +
    xh*wl, dropping the ~0.13% xl*wl term): 12 fp8 DoubleRow matmuls at
    K=256/instr and 0.5 cycles/row vs 8 fp32r matmuls at 1.0 — 1.33x fewer
    PE cycles, 4x less weight DMA. Final rel err ~7e-4 (gate 2e-2).
  - k/v biases ride the same PSUM group as one extra DoubleRow matmul
    (all-zero stationary except partition0 = 1, moving = [16b_hi|16b_lo]);
    q/out biases via ACT bias-ptr / Pool eviction add.

Layouts (no transposes on device): x feature-major chunk-blocked
[p, chunk, tok], k/v moving weights [p, half*8+chunk, 512], q stationary
weights [p, chunk, dout]; phi_k/vg/attn in fp16 (2x/4x DVE modes), phi_q
f32r (pairs with f32r ksum/kv in the verifier). Stage-2 tail: z with EPS
folded in as a K=1 matmul, 1/z broadcast per pair via a [16,128] selector
matmul reading r_sb directly (no copies), phi_q pre-scaled by it so the
kv matmul emits attn. The mask is exact but free: folded into the ksum
lhsT (mask columns) and the vg epilogue scalar, never applied to phi_k.

Scheduling: no DMA ever issues from the ACT queue (engine-issue costs
~1us and stalls the sigmoid chain); k weights stream on the Pool queue at
t=0 in half tiles ordered by first use, v/q/o weights trickle from inside
the 1a loop, x tiles and small copies ride the free SP queue. ksum runs
three tiles late, kv two, exp batches pair tiles (halves ACT table
loads), the stage-2 tail runs one chunk late (two at ch2, giving the
AllReduce a full extra chunk of cover), and pq holds 4 PSUM bufs so the
next chunk's matmuls ride through the exp-table reload.
"""
import numpy as np
import ml_dtypes
from contextlib import ExitStack

import concourse.mybir as mybir
import concourse.tile as tile
from concourse import bacc
from concourse.bass_utils import run_bass_kernel_spmd

F32 = mybir.dt.float32
F32R = mybir.dt.float32r
FP16 = mybir.dt.float16
FP8 = mybir.dt.float8e4
E4NP = ml_dtypes.float8_e4m3
ACTF = mybir.ActivationFunctionType
ALU = mybir.AluOpType
DR = mybir.MatmulPerfMode.DoubleRow

B, S, D, H = 4, 4096, 1024, 16
DK = D // H          # 64
EPS = 1e-6
NCORES = 8
T = B * S // NCORES  # 2048 tokens per core
P = 128
NM = T // P          # 16 token tiles
NCD = D // P         # 8 d-chunks
CH = 256             # stage-2 token chunk
NCH = T // CH        # 8 chunks
WS = 16.0            # weight pre-scale (folded back via ACT scale=1/WS)
GROUPS = [[0, 1], [2, 3], [4, 5], [6, 7]]


def build(single_core=False, stages="12"):
    nc = bacc.Bacc("TRN2", target_bir_lowering=False, debug=False,
                   num_devices=1 if single_core else NCORES)
    dt_in = {}

    def inp(name, shape, dt=F32):
        dt_in[name] = nc.dram_tensor(name, shape, dt, kind="ExternalInput").ap()

    for nm in ("xq", "xk", "xv"):
        for lvl in "hl":
            inp(f"{nm}8{lvl}", [P, T * NCD], FP8)
    for proj in ("k", "v"):
        for w in ("1", "2"):
            for lvl in "hl":
                inp(f"w{proj}{w}8{lvl}", [P, 2 * NCD * 512], FP8)
            inp(f"{proj}b{w}8", [P, 2 * D], FP8)
    for w in ("1", "2"):
        for lvl in "hl":
            inp(f"wq{w}8{lvl}", [P, NCD * D], FP8)
    inp("wo16", [P, NCD * D], mybir.dt.bfloat16)
    inp("onesb8", [P, 2 * P], FP8)
    inp("bq1c", [P, NCD]); inp("bq2c", [P, NCD])
    inp("bor", [P, D])
    inp("maskp16", [P, NM], FP16)   # mask columns: ksum lhsT (Sum mask*phi_k)
    inp("maskd16", [P, NM])         # mask/WS columns: vg scalar ptr
    inp("zeros16", [P, H])
    inp("blockmask", [H, NCD * P], mybir.dt.bfloat16)
    inp("ones16r", [1, H])
    inp("epsrow", [1, CH])
    out = nc.dram_tensor("out", [T, D], F32, kind="ExternalOutput").ap()

    with tile.TileContext(nc) as tc:
        _emit(nc, tc, dt_in, out, single_core, stages)
    nc.compile()
    return nc


def _emit(nc, tc, dt, out, single_core, stages="12"):
    def mm(psum, lhsT, rhs, start, stop, **kw):
        nc.tensor.matmul(psum, lhsT, rhs, start=start, stop=stop, **kw)

    has1 = "1" in stages
    has2 = "2" in stages

    with ExitStack() as st0:
        const = st0.enter_context(tc.tile_pool(name="const", bufs=1))
        dram = st0.enter_context(tc.tile_pool(name="dram", bufs=1, space="DRAM"))
        kvres = st0.enter_context(tc.tile_pool(name="kvres", bufs=1))
        kvstage_ctx = ExitStack()
        kvstage = kvstage_ctx.enter_context(tc.tile_pool(name="kvstage", bufs=1))

        # ---- constants (issued first, Pool queue) ----
        maskc = const.tile([P, NM], FP16, tag="maskc", name="maskc")
        nc.sync.dma_start(maskc[:], dt["maskp16"][:])
        maskd = const.tile([P, NM], F32, tag="maskd", name="maskd")
        nc.sync.dma_start(maskd[:], dt["maskd16"][:])
        bcol = {}
        for nm in ("bq1", "bq2"):
            bcol[nm] = const.tile([P, NCD], F32, tag=f"col_{nm}", name=f"col_{nm}")
            nc.sync.dma_start(bcol[nm][:], dt[nm + "c"][:])
        blockm = const.tile([H, NCD * P], mybir.dt.bfloat16, tag="blockm",
                            name="blockm")
        nc.sync.dma_start(blockm[:], dt["blockmask"][:])
        ones16 = const.tile([1, H], F32R, tag="ones16", name="ones16")
        nc.sync.dma_start(ones16[:], dt["ones16r"][:].bitcast(F32R))
        epsr = const.tile([1, CH], F32R, tag="epsr", name="epsr")
        nc.sync.dma_start(epsr[:], dt["epsrow"][:].bitcast(F32R))
        brep_o = const.tile([P, D], F32, tag="bor", name="bor")

        if not has1:
            kv_acc = [kvstage.tile([64, 512], F32, tag=f"kv_acc{i}",
                                   name=f"kv_acc{i}") for i in range(2)]
            for i in range(2):
                nc.any.memset(kv_acc[i][:], 1.0)
            cc_ks_sb = kvstage.tile([1, D], F32, tag="cc_ks_sb", name="cc_ks_sb")
            nc.any.memset(cc_ks_sb[:], 1.0)

        # pool creation order is LIFO-close order: phik (closes before stage
        # 2) before wv (closes after 1b) before wk (closes after 1a). DMA
        # issue priority is independent: k weights on the ACT queue, v/q/o
        # weights on the Pool queue, all at t=0.
        st1 = st0.enter_context(ExitStack())
        phik_pool = st1.enter_context(tc.tile_pool(name="phik", bufs=1))
        phi_k = [phik_pool.tile([P, D], FP16, tag=f"phik_{m}", name=f"phik_{m}")
                 for m in range(NM)] if has1 else []

        st_v = st0.enter_context(ExitStack())
        wvp = st_v.enter_context(tc.tile_pool(name="wv", bufs=1))
        st_k = st0.enter_context(ExitStack())
        wkp = st_k.enter_context(tc.tile_pool(name="wk", bufs=1))
        wqp = st0.enter_context(tc.tile_pool(name="wq", bufs=1, side="right"))
        xqp = st0.enter_context(tc.tile_pool(name="xq", bufs=3, side="right"))
        xq_tiles = {}

        def issue_xq(ch):
            xq_h = xqp.tile([P, NCD, CH], FP8, tag="xqh", name="xqh")
            xq_l = xqp.tile([P, NCD, CH], FP8, tag="xql", name="xql")
            nc.sync.dma_start(xq_h[:], dt["xq8h"][:, ch * 2048:(ch + 1) * 2048])
            nc.sync.dma_start(xq_l[:], dt["xq8l"][:, ch * 2048:(ch + 1) * 2048])
            xq_tiles[ch] = (xq_h, xq_l)

        # k weights in half-tile DMAs: the n=0 hi halves that gate tile 0
        # issue first, then bias tiles, then the rest
        wk_sb, kb_sb = {}, {}
        for w in ("1", "2"):
            for lvl in "hl":
                wk_sb[w, lvl] = wkp.tile([P, 2 * NCD, 512], FP8,
                                         tag=f"wk{w}{lvl}", name=f"wk{w}{lvl}")
        for w in ("1", "2"):
            kb_sb[w] = wkp.tile([P, 2, D], FP8, tag=f"kb{w}", name=f"kb{w}")
        onesb = const.tile([P, 2, P], FP8, tag="onesb", name="onesb")

        def _wk_dma(w, lvl, n):
            nc.gpsimd.dma_start(
                wk_sb[w, lvl][:, n * NCD:(n + 1) * NCD, :],
                dt[f"wk{w}8{lvl}"][:, n * NCD * 512:(n + 1) * NCD * 512])

        _wk_dma("1", "h", 0)
        _wk_dma("2", "h", 0)
        nc.gpsimd.dma_start(onesb[:], dt["onesb8"][:])
        for w in ("1", "2"):
            nc.gpsimd.dma_start(kb_sb[w][:], dt[f"kb{w}8"][:])
        _wk_dma("1", "h", 1)
        _wk_dma("2", "h", 1)
        for n in range(2):
            _wk_dma("1", "l", n)
            _wk_dma("2", "l", n)

        # v/q/o weights are not needed until t~100us+: queue their DMAs and
        # trickle them from inside the 1a loop so they don't steal DMA
        # bandwidth from the k weights + xk stream that gate early PE work
        deferred_dmas = []
        wv_sb, vb_sb = {}, {}
        for w in ("1", "2"):
            for lvl in "hl":
                t = wvp.tile([P, 2 * NCD, 512], FP8, tag=f"wv{w}{lvl}",
                             name=f"wv{w}{lvl}")
                deferred_dmas.append((t[:], dt[f"wv{w}8{lvl}"][:]))
                wv_sb[w, lvl] = t
            vb_sb[w] = wvp.tile([P, 2, D], FP8, tag=f"vb{w}", name=f"vb{w}")
            deferred_dmas.append((vb_sb[w][:], dt[f"vb{w}8"][:]))
        wq_sb = {}
        for w in ("1", "2"):
            for lvl in "hl":
                t = wqp.tile([P, NCD, D], FP8, tag=f"wq{w}{lvl}",
                             name=f"wq{w}{lvl}")
                deferred_dmas.append((t[:], dt[f"wq{w}8{lvl}"][:]))
                wq_sb[w, lvl] = t
        wo_sb = wqp.tile([P, NCD, D], mybir.dt.bfloat16, tag="wo", name="wo")
        deferred_dmas.append((wo_sb[:], dt["wo16"][:]))
        deferred_dmas.append((brep_o[:], dt["bor"][:]))
        if not has1:
            for dst, src in deferred_dmas:
                nc.gpsimd.dma_start(dst, src)
            deferred_dmas = []

        def glu_pair(p1, p2, x_h, x_l, w_sb, b_sb, n):
            """Two 13-matmul DoubleRow groups, stream-major: the wh-only
            streams (xh+xl) run first for both groups so the wl weight
            tiles can arrive late in the DMA order; the wl streams share
            each xh stationary chunk across p1/p2. p1 closes before p2 so
            its sigmoid overlaps p2's tail."""
            for w, psum in (("1", p1), ("2", p2)):
                for xs in (x_h, x_l):
                    for c in range(4):
                        cs = slice(n * NCD + 2 * c, n * NCD + 2 * c + 2)
                        mm(psum[:], xs[:, 2 * c:2 * c + 2, :],
                           w_sb[w, "h"][:, cs, :],
                           start=(xs is x_h and c == 0), stop=False,
                           perf_mode=DR)
            for c in range(4):
                stat = x_h[:, 2 * c:2 * c + 2, :]
                cs = slice(n * NCD + 2 * c, n * NCD + 2 * c + 2)
                for w, psum in (("1", p1), ("2", p2)):
                    mm(psum[:], stat, w_sb[w, "l"][:, cs, :],
                       start=False, stop=False, perf_mode=DR)
            for w, psum in (("1", p1), ("2", p2)):
                mm(psum[:], onesb[:], b_sb[w][:, :, n * 512:(n + 1) * 512],
                   start=False, stop=True, perf_mode=DR)

        # ================= stage 1a: k projection -> phi_k, ksum ============
        with ExitStack() as st1a:
            xkp = st1a.enter_context(tc.tile_pool(name="xk", bufs=4))
            t1a = st1a.enter_context(tc.tile_pool(name="t1a", bufs=2))
            pk1p = st1a.enter_context(tc.tile_pool(name="pk1", bufs=3, space="PSUM"))
            pk2p = st1a.enter_context(tc.tile_pool(name="pk2", bufs=3, space="PSUM"))
            pksp = st1a.enter_context(tc.tile_pool(name="pks", bufs=1, space="PSUM"))
            psum_ks = [pksp.tile([1, 512], F32, tag=f"ks{i}", name=f"ks{i}")
                       for i in range(2)]

            kq = []

            def ksum_tail(m):
                # lhsT = mask column: ksum = Sum_s mask_s * phi_k_s (the mask
                # is applied here and on vg, never on phi_k itself)
                for i in range(2):
                    mm(psum_ks[i][:], maskc[:, m:m + 1],
                       phi_k[m][:, i * 512:(i + 1) * 512],
                       start=(m == 0), stop=(m == NM - 1))

            for m in range(NM if has1 else 0):
                xk_h = xkp.tile([P, NCD, P], FP8, tag="xkh", name="xkh")
                xk_l = xkp.tile([P, NCD, P], FP8, tag="xkl", name="xkl")
                nc.sync.dma_start(xk_h[:], dt["xk8h"][:, m * D:(m + 1) * D])
                nc.sync.dma_start(xk_l[:], dt["xk8l"][:, m * D:(m + 1) * D])
                if m >= 5:
                    for dst, src in deferred_dmas[2 * (m - 5):2 * (m - 4)]:
                        nc.gpsimd.dma_start(dst, src)
                for n in range(2):
                    p1 = pk1p.tile([P, 512], F32, tag="pk1", name="pk1")
                    p2 = pk2p.tile([P, 512], F32, tag="pk2", name="pk2")
                    glu_pair(p1, p2, xk_h, xk_l, wk_sb, kb_sb, n)
                    a1 = t1a.tile([P, 512], FP16, tag="a1", name="a1")
                    nc.scalar.activation(a1[:], p1[:], ACTF.Sigmoid,
                                         scale=1.0 / WS)
                    g1 = t1a.tile([P, 512], FP16, tag="g1", name="g1")
                    nc.vector.scalar_tensor_tensor(g1[:], p1[:], 1.0 / WS, a1[:],
                                                   ALU.mult, ALU.mult)
                    kg = t1a.tile([P, 512], FP16, tag="kg", name="kg", bufs=4)
                    nc.vector.scalar_tensor_tensor(kg[:], p2[:], 1.0 / WS, g1[:],
                                                   ALU.mult, ALU.mult)
                    tmin = t1a.tile([P, 512], FP16, tag="tmin", name="tmin", bufs=4)
                    nc.vector.tensor_scalar_min(tmin[:], kg[:], 0.0)
                    kq.append((m, n, kg, tmin))
                if m % 2 == 1:  # Exp batch across the tile pair (2 table
                    for bm, bn, kg, tmin in kq:  # switches per 2 tiles)
                        ns = slice(bn * 512, (bn + 1) * 512)
                        texp = t1a.tile([P, 512], FP16, tag="texp", name="texp")
                        nc.scalar.activation(texp[:], tmin[:], ACTF.Exp)
                        trel = t1a.tile([P, 512], FP16, tag="trel", name="trel")
                        nc.vector.tensor_scalar_max(trel[:], kg[:], 0.0)
                        # phi_k = exp(min(kg,0)) + relu(kg)  (mask on vg/ksum)
                        nc.gpsimd.tensor_tensor(phi_k[bm][:, ns], texp[:],
                                                trel[:], ALU.add)
                    kq = []
                if m >= 2:
                    ksum_tail(m - 2)
            if has1:
                for mt in (NM - 2, NM - 1):
                    ksum_tail(mt)
                cc_ks_sb = kvstage.tile([1, D], F32, tag="cc_ks_sb", name="cc_ks_sb")
                for i in range(2):
                    nc.vector.tensor_copy(cc_ks_sb[0:1, i * 512:(i + 1) * 512],
                                          psum_ks[i][:])
        st_k.close()  # frees k weights
        if has2:
            issue_xq(0)
            issue_xq(1)

        # ============== stage 1b: v projection + kv accumulation ============
        with ExitStack() as st1b:
            xvp = st1b.enter_context(tc.tile_pool(name="xv", bufs=6))
            t1b = st1b.enter_context(tc.tile_pool(name="t1b", bufs=3))
            vgp = st1b.enter_context(tc.tile_pool(name="vgp", bufs=4))
            pv1p = st1b.enter_context(tc.tile_pool(name="pv1", bufs=2, space="PSUM"))
            pv2p = st1b.enter_context(tc.tile_pool(name="pv2", bufs=2, space="PSUM"))
            pkvp = st1b.enter_context(tc.tile_pool(name="pkv", bufs=1, space="PSUM"))
            if has1:
                psum_kv = [pkvp.tile([64, 512], F32, tag=f"pkv{i}", name=f"pkv{i}")
                           for i in range(2)]

            def kv_tail(m, vg_m):
                # one global accumulation group per bank: start only on the very
                # first matmul (has_written is per element)
                for h in range(H):
                    hs = slice(h * DK, (h + 1) * DK)
                    first = (m == 0 and h % 8 == 0)
                    last = (m == NM - 1 and h % 8 == 7)
                    nc.tensor.matmul(
                        psum_kv[h // 8][0:64, (h % 8) * DK:(h % 8 + 1) * DK],
                        phi_k[m][:, hs], vg_m[:, hs],
                        start=first, stop=last,
                        skip_group_check=not (first or last))

            vg_hist = []
            for m in range(NM if has1 else 0):
                xv_h = xvp.tile([P, NCD, P], FP8, tag="xvh", name="xvh")
                xv_l = xvp.tile([P, NCD, P], FP8, tag="xvl", name="xvl")
                nc.sync.dma_start(xv_h[:], dt["xv8h"][:, m * D:(m + 1) * D])
                nc.sync.dma_start(xv_l[:], dt["xv8l"][:, m * D:(m + 1) * D])
                vg = vgp.tile([P, D], FP16, tag="vg", name="vg")
                for n in range(2):
                    ns = slice(n * 512, (n + 1) * 512)
                    p1 = pv1p.tile([P, 512], F32, tag="pv1", name="pv1")
                    p2 = pv2p.tile([P, 512], F32, tag="pv2", name="pv2")
                    glu_pair(p1, p2, xv_h, xv_l, wv_sb, vb_sb, n)
                    a1 = t1b.tile([P, 512], FP16, tag="va1", name="va1")
                    nc.scalar.activation(a1[:], p1[:], ACTF.Sigmoid,
                                         scale=1.0 / WS)
                    g1 = t1b.tile([P, 512], FP16, tag="vg1", name="vg1")
                    nc.vector.scalar_tensor_tensor(g1[:], p1[:], 1.0 / WS, a1[:],
                                                   ALU.mult, ALU.mult)
                    # mask/WS ptr: vg = silu(t1) * t2 * mask (mask lives here)
                    nc.vector.scalar_tensor_tensor(vg[:, ns], p2[:],
                                                   maskd[:, m:m + 1],
                                                   g1[:], ALU.mult, ALU.mult)
                vg_hist.append(vg)
                if m >= 2:
                    kv_tail(m - 2, vg_hist[m - 2])
            if has1:
                kv_tail(NM - 2, vg_hist[NM - 2])
                kv_tail(NM - 1, vg_hist[NM - 1])
                kv_acc = [kvstage.tile([64, 512], F32, tag=f"kv_acc{i}",
                                       name=f"kv_acc{i}") for i in range(2)]
                for i in range(2):
                    nc.vector.tensor_copy(kv_acc[i][:], psum_kv[i][:])
        st_v.close()
        st1.close()  # frees phi_k SBUF before stage 2

        # ============ collective: pair AllReduce of kv + ksum ============
        cc_in = dram.tile([130, 512], F32)
        cc_out = dram.tile([130, 512], F32)
        nc.sync.dma_start(cc_in[0:64, :], kv_acc[0][:])
        nc.sync.dma_start(cc_in[64:128, :], kv_acc[1][:])
        nc.sync.dma_start(cc_in[128:130, :], cc_ks_sb[:])
        kvstage_ctx.close()
        if single_core:
            nc.sync.dma_start(cc_out[:], cc_in[:])
        else:
            nc.gpsimd.collective_compute(
                "AllReduce", ALU.add, replica_groups=GROUPS,
                ins=[cc_in.opt()], outs=[cc_out.opt()])

        # reduced kv -> pair-packed sbuf tile; ksum -> block-diag lhsT tiles
        kv_pairs = kvres.tile([P, 512], F32R, tag="kv_pairs", name="kv_pairs")
        for h in range(H):
            r0 = 0 if h < 8 else 64
            eng = (nc.sync, nc.gpsimd, nc.sync, nc.gpsimd)[h % 4]
            eng.dma_start(
                kv_pairs[(h % 2) * 64:(h % 2) * 64 + 64,
                         (h // 2) * DK:(h // 2 + 1) * DK],
                cc_out[r0:r0 + 64, (h % 8) * DK:(h % 8 + 1) * DK].bitcast(F32R))
        ksum_bd = []
        for c in range(NCD):
            bd = kvres.tile([P, H], F32R, tag=f"bd{c}", name=f"bd{c}")
            (nc.sync, nc.gpsimd, nc.sync, nc.gpsimd)[c % 4].dma_start(
                bd[:], dt["zeros16"][:].bitcast(F32R))
            # ksum[d] lives at cc_out[128 + d // 512, d % 512]
            for half, cs in ((0, 2 * c), (64, 2 * c + 1)):
                d0 = c * P + half
                (nc.sync, nc.gpsimd, nc.sync, nc.gpsimd)[(c + half // 64) % 4].dma_start(
                    bd[half:half + 64, cs:cs + 1],
                    cc_out[128 + d0 // 512:129 + d0 // 512,
                           d0 % 512:d0 % 512 + 64].bitcast(F32R))
            ksum_bd.append(bd)

        # ============ stage 2: q -> phi_q -> z -> attn -> out ============
        with ExitStack() as st2:
            phiqp = st2.enter_context(tc.tile_pool(name="phiq", bufs=3))
            attnp = st2.enter_context(tc.tile_pool(name="attn", bufs=3))
            t2 = st2.enter_context(tc.tile_pool(name="t2", bufs=4))
            tz = st2.enter_context(tc.tile_pool(name="tz", bufs=2))
            osbp = st2.enter_context(tc.tile_pool(name="osb", bufs=3))
            # 8 PSUM banks: pq (p1|p2 packed) 4, pn 2, po (pz/pr/po) 2
            pqp = st2.enter_context(tc.tile_pool(name="pq", bufs=4, space="PSUM"))
            pnp = st2.enter_context(tc.tile_pool(name="pn", bufs=2, space="PSUM"))
            pop = st2.enter_context(tc.tile_pool(name="po", bufs=2, space="PSUM"))

            def tail_head(phi_q):
                pzt = pop.tile([P, 512], F32, tag="po", name="pzt")
                pz = pzt[0:H, 0:CH]
                mm(pz, ones16[:], epsr[:], start=True, stop=False)  # +EPS
                for c in range(NCD):
                    mm(pz, ksum_bd[c][:], phi_q[c][:],
                       start=False, stop=(c == NCD - 1))
                r_sb = tz.tile([H, CH], mybir.dt.bfloat16, tag="r_sb",
                               name="r_sb")
                with nc.allow_low_precision(reason="1/z broadcast tolerates bf16"):
                    nc.vector.reciprocal(r_sb[:], pz)
                attn = [attnp.tile([P, CH], FP16, tag=f"attn{c}", name=f"attn{c}")
                        for c in range(NCD)]
                return r_sb, attn

            def tail_pair(phi_q, r_sb, attn, pair):
                # DVE reads at most one PSUM input: scale phi_q by the
                # broadcast reciprocal first (SBUF x PSUM), then the kv
                # matmuls yield attn directly in PSUM. Both pn halves sit at
                # partition base 0 (base-64 matmul outputs are invalid ISA);
                # the evictions shift head 2p+1 up to partitions 64:128.
                # The [16,128] selector block reads r_sb directly (no copies).
                t = pop.tile([P, 512], F32, tag="po", name="prt")
                pr = t[:, 0:CH]
                mm(pr, blockm[:, pair * P:(pair + 1) * P], r_sb[:, :],
                   start=True, stop=True)
                pqr = tz.tile([P, CH], F32R, tag="pqr", name="pqr", bufs=2)
                nc.vector.tensor_tensor(pqr[:], phi_q[pair][:], pr, ALU.mult)
                # separate PSUM tiles per head: mixing tile-position rows
                # (0 vs 64) inside one PSUM tile crashes the runtime
                cs = slice(pair * DK, (pair + 1) * DK)
                pna = pnp.tile([64, CH], F32, tag="pn", name="pna")
                mm(pna[:], kv_pairs[0:64, cs], pqr[0:64, :],
                   start=True, stop=True)
                pnb = pnp.tile([64, CH], F32, tag="pn", name="pnb")
                mm(pnb[:], kv_pairs[64:128, cs], pqr[64:128, :],
                   start=True, stop=True)
                nc.scalar.activation(attn[pair][0:64, :], pna[:], ACTF.Copy)
                nc.vector.tensor_copy(attn[pair][64:128, :], pnb[:])

            def tail_out(ch, attn):
                for mt in range(CH // P):
                    o_sb = osbp.tile([P, D], F32, tag="o_sb", name="o_sb")
                    for n in range(2):
                        ns = slice(n * 512, (n + 1) * 512)
                        po = pop.tile([P, 512], F32, tag="po", name="po")
                        for c in range(NCD):
                            mm(po[:], attn[c][:, mt * P:(mt + 1) * P],
                               wo_sb[:, c, ns], start=(c == 0), stop=(c == NCD - 1))
                        nc.vector.tensor_tensor(o_sb[:, ns], po[:],
                                                brep_o[:, ns], ALU.add)
                    row0 = ch * CH + mt * P
                    nc.gpsimd.dma_start(out[row0:row0 + P, :], o_sb[:])

            # sub-stage bisection: stages '2a' = GLU only, '2b' = +tail_head,
            # '2c' = +tail_pair, '2'/'12' = everything
            sub = stages[stages.index("2") + 1:] if "2" in stages else ""
            do_head = sub in ("", "b", "c")
            do_pair = sub in ("", "c")
            do_out = sub == ""
            pending = []
            for ch in range(NCH if has2 else 0):
                if ch + 2 < NCH:
                    issue_xq(ch + 2)
                xq_h, xq_l = xq_tiles.pop(ch)
                # f32r (not fp16): the BIR verifier requires f32r matmul
                # operands to pair with f32r (z/pn read these against
                # f32r ksum_bd/kv_pairs); moving f32r at N=256 is still
                # 1 cycle/row.
                phi_q = [phiqp.tile([P, CH], F32R, tag=f"phiq{mc}",
                                    name=f"phiq{mc}") for mc in range(NCD)]
                # no tail at ch1: the collective gets a full extra chunk of
                # GLU cover; ch2 drains both pending tails
                tails = pending if ch >= 2 else []
                heads = [(p_ch, p_phi, *tail_head(p_phi))
                         for p_ch, p_phi in tails] if do_head else []
                qgs, qtmins = [], []
                for mc in range(NCD):
                    ms = slice(mc * P, (mc + 1) * P)
                    t_q = pqp.tile([P, 512], F32, tag="pq", name="pq")
                    p1, p2 = t_q[:, 0:CH], t_q[:, CH:2 * CH]
                    for psum, w in ((p1, "1"), (p2, "2")):
                        # xh/xl share each wh stationary chunk (fewer Ldweights)
                        for c in range(4):
                            stat = wq_sb[w, "h"][:, 2 * c:2 * c + 2, ms]
                            mm(psum, stat, xq_h[:, 2 * c:2 * c + 2, :],
                               start=(c == 0), stop=False, perf_mode=DR)
                            mm(psum, stat, xq_l[:, 2 * c:2 * c + 2, :],
                               start=False, stop=False, perf_mode=DR)
                        for c in range(4):
                            mm(psum, wq_sb[w, "l"][:, 2 * c:2 * c + 2, ms],
                               xq_h[:, 2 * c:2 * c + 2, :],
                               start=False, stop=(c == 3), perf_mode=DR)
                    a1 = t2.tile([P, CH], FP16, tag="qa1", name="qa1")
                    nc.scalar.activation(a1[:], p1, ACTF.Sigmoid,
                                         bias=bcol["bq1"][:, mc:mc + 1],
                                         scale=1.0 / WS)
                    t1b = t2.tile([P, CH], FP16, tag="qt1", name="qt1")
                    nc.scalar.activation(t1b[:], p1, ACTF.Identity,
                                         bias=bcol["bq1"][:, mc:mc + 1],
                                         scale=1.0 / WS)
                    t2b = t2.tile([P, CH], FP16, tag="qt2", name="qt2")
                    nc.scalar.activation(t2b[:], p2, ACTF.Identity,
                                         bias=bcol["bq2"][:, mc:mc + 1],
                                         scale=1.0 / WS)
                    s1 = t2.tile([P, CH], FP16, tag="qs1", name="qs1")
                    nc.vector.tensor_tensor(s1[:], t1b[:], a1[:], ALU.mult)
                    qg = t2.tile([P, CH], FP16, tag="qg", name="qg", bufs=NCD)
                    nc.vector.tensor_tensor(qg[:], t2b[:], s1[:], ALU.mult)
                    tmin = t2.tile([P, CH], FP16, tag="qtmin", name="qtmin",
                                   bufs=NCD)
                    nc.vector.tensor_scalar_min(tmin[:], qg[:], 0.0)
                    qgs.append(qg)
                    qtmins.append(tmin)
                    if do_pair:
                        for p_ch, p_phi, p_rsb, p_attn in heads:
                            tail_pair(p_phi, p_rsb, p_attn, mc)
                for mc in range(NCD):  # Exp batch + phi assembly
                    texp = t2.tile([P, CH], FP16, tag="qtexp", name="qtexp")
                    nc.scalar.activation(texp[:], qtmins[mc][:], ACTF.Exp)
                    # phi_q = relu(qg) + exp(min(qg,0))
                    nc.vector.scalar_tensor_tensor(phi_q[mc][:], qgs[mc][:], 0.0,
                                                   texp[:], ALU.max, ALU.add)
                if do_out:
                    for p_ch, p_phi, p_rsb, p_attn in heads:
                        tail_out(p_ch, p_attn)
                pending = [t for t in pending if t[0] not in
                           {h[0] for h in heads}]
                pending.append((ch, phi_q))
            if has2 and do_head:
                # drain: out-proj groups (on the free pq banks) trail the
                # attn pairs by one, so PE never waits on an eviction
                # before the next pair's matmuls
                for p_ch, p_phi in pending:
                    p_rsb, p_attn = tail_head(p_phi)
                    if not do_pair:
                        continue
                    units = []
                    if do_out:
                        for mt in range(CH // P):
                            for n in range(2):
                                po = pqp.tile([P, 512], F32, tag="pq",
                                              name="pod")
                                units.append((mt, n, po))

                    def drain_po(c):
                        for mt, n, po in units:
                            mm(po[:, 0:512],
                               p_attn[c][:, mt * P:(mt + 1) * P],
                               wo_sb[:, c, n * 512:(n + 1) * 512],
                               start=(c == 0), stop=(c == NCD - 1))

                    for pair in range(NCD):
                        tail_pair(p_phi, p_rsb, p_attn, pair)
                        if pair >= 1:
                            drain_po(pair - 1)
                    drain_po(NCD - 1)
                    for mt in range(CH // P if do_out else 0):
                        o_sb = osbp.tile([P, D], F32, tag="o_sb", name="o_sb")
                        for mt2, n, po in units:
                            if mt2 == mt:
                                nc.vector.tensor_tensor(
                                    o_sb[:, n * 512:(n + 1) * 512],
                                    po[:, 0:512],
                                    brep_o[:, n * 512:(n + 1) * 512], ALU.add)
                        row0 = p_ch * CH + mt * P
                        nc.gpsimd.dma_start(out[row0:row0 + P, :], o_sb[:])


_CACHE = {}


def _get_nc(single_core=False):
    key = bool(single_core)
    if key not in _CACHE:
        _CACHE[key] = build(single_core)
    return _CACHE[key]


def _split8(a):
    hi = a.astype(E4NP)
    lo = (a - hi.astype(np.float32)).astype(E4NP)
    return hi, lo


def _pack_x_tiles(x, tok):
    """x [T, D] f32 -> hi/lo [128, T*8] fp8, cols = blk*(8*tok) + c*tok + t."""
    xf = np.ascontiguousarray(x.T)                       # [D, T]
    a = xf.reshape(NCD, P, T // tok, tok)                # [c, p, blk, t]
    a = a.transpose(1, 2, 0, 3).reshape(P, T * NCD)      # [p, blk*c*t]
    return _split8(np.ascontiguousarray(a))


def _pack_w_moving(w):
    """W [dout, din] -> hi/lo [128, 8192] fp8, cols = n*4096 + c*512 + j."""
    wT = (WS * np.asarray(w, np.float32)).T              # [din, dout]
    a = wT.reshape(NCD, P, 2, 512).transpose(1, 2, 0, 3).reshape(P, 2 * NCD * 512)
    return _split8(np.ascontiguousarray(a))


def _pack_w_stationary(w):
    """W [dout, din] -> hi/lo [128, 8192] fp8, cols = c*1024 + m."""
    wT = (WS * np.asarray(w, np.float32)).T
    a = wT.reshape(NCD, P, D).transpose(1, 0, 2).reshape(P, NCD * D)
    return _split8(np.ascontiguousarray(a))


def _pack_bias(b):
    """b [D] -> [128, 2048] fp8: row0 = [16b_hi | 16b_lo], rest zero."""
    bh, bl = _split8(WS * np.asarray(b, np.float32))
    a = np.zeros((P, 2 * D), E4NP)
    a[0, :D] = bh
    a[0, D:] = bl
    return a


def make_in_maps(inputs):
    f = np.float32
    q = np.asarray(inputs["query"], f).reshape(B * S, D)
    k = np.asarray(inputs["key"], f).reshape(B * S, D)
    v = np.asarray(inputs["value"], f).reshape(B * S, D)
    mask = np.asarray(inputs["mask"], f).reshape(B * S)

    onesb = np.zeros((P, 2 * P), E4NP)
    onesb[0, :] = 1.0
    blockmask = np.zeros((H, NCD * P), ml_dtypes.bfloat16)
    for pr_ in range(NCD):
        blockmask[2 * pr_, pr_ * P:pr_ * P + 64] = 1.0
        blockmask[2 * pr_ + 1, pr_ * P + 64:(pr_ + 1) * P] = 1.0

    shared = {
        "onesb8": onesb,
        "bq1c": np.ascontiguousarray(np.asarray(inputs["q_b1"], f).reshape(NCD, P).T),
        "bq2c": np.ascontiguousarray(np.asarray(inputs["q_b2"], f).reshape(NCD, P).T),
        "bor": np.tile(np.asarray(inputs["out_b"], f)[None, :], (P, 1)),
        "zeros16": np.zeros((P, H), f),
        "blockmask": blockmask,
        "ones16r": np.ones((1, H), f),
        "epsrow": np.full((1, CH), EPS, f),
        "wo16": np.ascontiguousarray(
            np.asarray(inputs["out_w"], f).T.reshape(NCD, P, D)
            .transpose(1, 0, 2).reshape(P, NCD * D).astype(ml_dtypes.bfloat16)),
    }
    for proj, w1k, b1k, w2k, b2k in (("k", "k_w1", "k_b1", "k_w2", "k_b2"),
                                     ("v", "v_w1", "v_b1", "v_w2", "v_b2")):
        for w, wk, bk in (("1", w1k, b1k), ("2", w2k, b2k)):
            hi, lo = _pack_w_moving(inputs[wk])
            shared[f"w{proj}{w}8h"] = hi
            shared[f"w{proj}{w}8l"] = lo
            shared[f"{proj}b{w}8"] = _pack_bias(inputs[bk])
    for w, wk in (("1", "q_w1"), ("2", "q_w2")):
        hi, lo = _pack_w_stationary(inputs[wk])
        shared[f"wq{w}8h"] = hi
        shared[f"wq{w}8l"] = lo

    in_maps = []
    for c in range(NCORES):
        sl = slice(c * T, (c + 1) * T)
        m = dict(shared)
        for nm, x in (("xq", q), ("xk", k), ("xv", v)):
            tok = CH if nm == "xq" else P
            hi, lo = _pack_x_tiles(x[sl], tok)
            m[f"{nm}8h"] = hi
            m[f"{nm}8l"] = lo
        mcol = np.ascontiguousarray(mask[sl].reshape(NM, P).T)
        m["maskp16"] = mcol.astype(np.float16)
        m["maskd16"] = np.ascontiguousarray(mcol / WS)
        in_maps.append(m)
    return in_maps


def kernel(**inputs):
    nc = _get_nc(False)
    in_maps = make_in_maps(inputs)
    res = run_bass_kernel_spmd(nc, in_maps, list(range(NCORES))).results
    outc = np.concatenate([res[c]["out"] for c in range(NCORES)], axis=0)
    return outc.reshape(B, S, D)



# revision 49
# speedup vs baseline: 1.0906x; 1.0906x over previous
"""MultiHeadLinearAttention Trainium2 kernel (8-core SPMD, fp8 DoubleRow GLU).

Sharding: 16384 tokens split across 8 cores (core c: batch c//2, sequence half
c%2). All projections/attention/out-proj are local; the only cross-core
dependency is the per-batch KV summary (kv [H,DK,DK] + ksum [D]) reduced via a
266KB pair-wise AllReduce, overlapped with early stage-2 compute.

Matmul scheme (the six GLU matmuls, ~86% of PE work):
  - host splits x and 16*W each into two fp8e4m3 levels (hi = fp8(a),
    lo = fp8(a - hi)); the x16 weight pre-scaling keeps the lo level inside
    e4m3's normal range (raw residuals of U(+-1/32) weights sit below the
    2^-9 denormal floor). The 1/16 is folded back via ACT's scale param.
  - each 1024-deep contraction runs as 3 DoubleRow streams (xh*wh + xl*wh +
    xh*wl, dropping the ~0.13% xl*wl term): 12 fp8 DoubleRow matmuls at
    K=256/instr and 0.5 cycles/row vs 8 fp32r matmuls at 1.0 — 1.33x fewer
    PE cycles, 4x less weight DMA. Final rel err ~7e-4 (gate 2e-2).
  - k/v biases ride the same PSUM group as one extra DoubleRow matmul
    (all-zero stationary except partition0 = 1, moving = [16b_hi|16b_lo]);
    q/out biases via ACT bias-ptr / Pool eviction add.

Layouts (no transposes on device): x feature-major chunk-blocked
[p, chunk, tok], k/v moving weights [p, half*8+chunk, 512], q stationary
weights [p, chunk, dout]; phi_k/vg/attn in fp16 (2x/4x DVE modes), phi_q
f32r (pairs with f32r ksum/kv in the verifier). Stage-2 tail: z with EPS
folded in as a K=1 matmul, 1/z broadcast per pair via a [16,128] selector
matmul reading r_sb directly (no copies), phi_q pre-scaled by it so the
kv matmul emits attn. The mask is exact but free: folded into the ksum
lhsT (mask columns) and the vg epilogue scalar, never applied to phi_k.

Scheduling: no DMA ever issues from the ACT queue (engine-issue costs
~1us and stalls the sigmoid chain); k weights stream on the Pool queue at
t=0 in half tiles ordered by first use, v/q/o weights trickle from inside
the 1a loop, x tiles and small copies ride the free SP queue. ksum runs
three tiles late, kv two, exp batches pair tiles (halves ACT table
loads), the stage-2 tail runs one chunk late (two at ch2, giving the
AllReduce a full extra chunk of cover), and pq holds 4 PSUM bufs so the
next chunk's matmuls ride through the exp-table reload.
"""
import numpy as np
import ml_dtypes
from contextlib import ExitStack

import concourse.mybir as mybir
import concourse.tile as tile
from concourse import bacc
from concourse.bass_utils import run_bass_kernel_spmd

F32 = mybir.dt.float32
F32R = mybir.dt.float32r
FP16 = mybir.dt.float16
FP8 = mybir.dt.float8e4
E4NP = ml_dtypes.float8_e4m3
ACTF = mybir.ActivationFunctionType
ALU = mybir.AluOpType
DR = mybir.MatmulPerfMode.DoubleRow

B, S, D, H = 4, 4096, 1024, 16
DK = D // H          # 64
EPS = 1e-6
NCORES = 8
T = B * S // NCORES  # 2048 tokens per core
P = 128
NM = T // P          # 16 token tiles
NCD = D // P         # 8 d-chunks
CH = 256             # stage-2 token chunk
NCH = T // CH        # 8 chunks
WS = 16.0            # weight pre-scale (folded back via ACT scale=1/WS)
LOGWS = float(np.log(WS))
ASC = 128.0          # attn fp8 eviction scale (attn ~1e-3; e4m3 needs O(0.1-4))
GROUPS = [[0, 1], [2, 3], [4, 5], [6, 7]]


def build(single_core=False, stages="12", debug_cc=False):
    nc = bacc.Bacc("TRN2", target_bir_lowering=False, debug=False,
                   num_devices=1 if single_core else NCORES)
    dt_in = {}

    def inp(name, shape, dt=F32):
        dt_in[name] = nc.dram_tensor(name, shape, dt, kind="ExternalInput").ap()

    for nm in ("xq", "xk", "xv"):
        for lvl in "hl":
            inp(f"{nm}8{lvl}", [P, T * NCD], FP8)
    for proj in ("k", "v"):
        for w in ("1", "2"):
            for lvl in "hl":
                inp(f"w{proj}{w}8{lvl}", [P, 2 * NCD * 512], FP8)
            inp(f"{proj}b{w}8", [P, 2 * D], FP8)
    for w in ("1", "2"):
        for lvl in "hl":
            inp(f"wq{w}8{lvl}", [P, NCD * D], FP8)
    for lvl in "hl":
        inp(f"wo8{lvl}", [P, 2 * NCD * 512], FP8)
    inp("onesb8", [P, 2 * P], FP8)
    inp("bq1c", [P, NCD]); inp("bq2c", [P, NCD])
    inp("bor", [P, D])
    inp("maskp16", [P, NM], FP16)   # mask columns: ksum moving operand
    inp("maskd16", [P, NM])         # mask/WS columns: vg scalar ptr
    inp("blockmask", [H, NCD * P], mybir.dt.bfloat16)
    inp("ones16r", [1, H])
    inp("epsrow", [1, CH])
    out = nc.dram_tensor("out", [T, D], F32, kind="ExternalOutput").ap()
    dbg = (nc.dram_tensor("dbg", [P, 1152], F32, kind="ExternalOutput").ap()
           if debug_cc else None)

    with tile.TileContext(nc) as tc:
        _emit(nc, tc, dt_in, out, single_core, stages, dbg)
    nc.compile()
    return nc


def _emit(nc, tc, dt, out, single_core, stages="12", dbg=None):
    def mm(psum, lhsT, rhs, start, stop, **kw):
        nc.tensor.matmul(psum, lhsT, rhs, start=start, stop=stop, **kw)

    has1 = "1" in stages
    has2 = "2" in stages

    with ExitStack() as st0:
        const = st0.enter_context(tc.tile_pool(name="const", bufs=1))
        dram = st0.enter_context(tc.tile_pool(name="dram", bufs=1, space="DRAM"))
        kvres = st0.enter_context(tc.tile_pool(name="kvres", bufs=1))

        # collective payload: [P, 1152] f32 — cols 0:1024 kv as 8 BLOCK-DIAG
        # [128,128] pair tiles (head 2p at rows/cols 0:64 of block p, head
        # 2p+1 at 64:128), cols 1024:1152 the block-diag ksum tiles. The
        # zeros AllReduce to zeros, so stage 2 gets its exact stationary
        # layouts back in two post-collective DMAs: the pn matmul runs one
        # [128,128] x [128,CH] per pair (half the matmuls/evictions of the
        # per-head [64,64] form).
        cc_in = dram.tile([P, 1152], F32)
        cc_out = dram.tile([P, 1152], F32)
        kvstk = kvres.tile([P, 16, 64], F32, tag="kvstk", name="kvstk")
        kvstb = kvres.tile([P, P], F32, tag="kvstb", name="kvstb")
        nc.vector.memset(kvstk[:], 0.0)
        nc.vector.memset(kvstb[:], 0.0)

        # pool creation order is LIFO-close order: phik (closes before stage
        # 2) before wv (closes after 1b) before wk (closes after 1a).
        st1 = st0.enter_context(ExitStack())
        phik_pool = st1.enter_context(tc.tile_pool(name="phik", bufs=1))
        phi_k = [phik_pool.tile([P, D], FP16, tag=f"phik_{m}", name=f"phik_{m}")
                 for m in range(NM)] if has1 else []

        st_v = st0.enter_context(ExitStack())
        wvp = st_v.enter_context(tc.tile_pool(name="wv", bufs=1))
        st_k = st0.enter_context(ExitStack())
        wkp = st_k.enter_context(tc.tile_pool(name="wk", bufs=1))
        st_xk = st0.enter_context(ExitStack())
        xkp = st_xk.enter_context(tc.tile_pool(name="xk", bufs=4))
        wqp = st0.enter_context(tc.tile_pool(name="wq", bufs=1, side="right"))
        xqp = st0.enter_context(tc.tile_pool(name="xq", bufs=2, side="right"))
        xq_tiles = {}

        def issue_xq(ch):
            xq_h = xqp.tile([P, NCD, CH], FP8, tag="xqh", name="xqh")
            xq_l = xqp.tile([P, NCD, CH], FP8, tag="xql", name="xql")
            nc.sync.dma_start(xq_h[:], dt["xq8h"][:, ch * 2048:(ch + 1) * 2048])
            nc.sync.dma_start(xq_l[:], dt["xq8l"][:, ch * 2048:(ch + 1) * 2048])
            xq_tiles[ch] = (xq_h, xq_l)

        xk_tiles = {}

        def issue_xk(m):
            xk_h = xkp.tile([P, NCD, P], FP8, tag="xkh", name="xkh")
            xk_l = xkp.tile([P, NCD, P], FP8, tag="xkl", name="xkl")
            nc.sync.dma_start(xk_h[:], dt["xk8h"][:, m * D:(m + 1) * D])
            nc.sync.dma_start(xk_l[:], dt["xk8l"][:, m * D:(m + 1) * D])
            xk_tiles[m] = (xk_h, xk_l)

        # x tile 0 gates the first Ldweights: issue it before everything
        # else on the SP queue
        if has1:
            issue_xk(0)
            issue_xk(1)

        # k weights in half-tile DMAs split across the Pool and DVE queues so
        # both first-use halves land in parallel; bias tiles next, then the
        # rest. Consts ride the DVE queue after the critical k weights.
        wk_sb, kb_sb = {}, {}
        for w in ("1", "2"):
            for lvl in "hl":
                wk_sb[w, lvl] = wkp.tile([P, 2 * NCD, 512], FP8,
                                         tag=f"wk{w}{lvl}", name=f"wk{w}{lvl}")
        for w in ("1", "2"):
            kb_sb[w] = wkp.tile([P, 2, D], FP8, tag=f"kb{w}", name=f"kb{w}")
        onesb = const.tile([P, 2, P], FP8, tag="onesb", name="onesb")

        def _wk_dma(w, lvl, n, eng=None):
            (eng or nc.gpsimd).dma_start(
                wk_sb[w, lvl][:, n * NCD:(n + 1) * NCD, :],
                dt[f"wk{w}8{lvl}"][:, n * NCD * 512:(n + 1) * NCD * 512])

        _wk_dma("1", "h", 0)
        _wk_dma("2", "h", 0)
        nc.gpsimd.dma_start(onesb[:], dt["onesb8"][:])
        for w in ("1", "2"):
            nc.gpsimd.dma_start(kb_sb[w][:], dt[f"kb{w}8"][:])
        _wk_dma("1", "l", 0)
        _wk_dma("2", "l", 0)
        _wk_dma("1", "h", 1)
        _wk_dma("2", "h", 1)
        _wk_dma("1", "l", 1)
        _wk_dma("2", "l", 1)

        # ---- constants (SP queue after the xk pre-issues; first use ~15us) --
        maskc = const.tile([P, NM], FP16, tag="maskc", name="maskc")
        nc.sync.dma_start(maskc[:], dt["maskp16"][:])
        maskd = const.tile([P, NM], F32, tag="maskd", name="maskd")
        nc.sync.dma_start(maskd[:], dt["maskd16"][:])
        bcol = {}
        for nm in ("bq1", "bq2"):
            bcol[nm] = const.tile([P, NCD], F32, tag=f"col_{nm}", name=f"col_{nm}")
            nc.sync.dma_start(bcol[nm][:], dt[nm + "c"][:])
        blockm = const.tile([H, NCD * P], mybir.dt.bfloat16, tag="blockm",
                            name="blockm")
        nc.sync.dma_start(blockm[:], dt["blockmask"][:])
        ones16 = const.tile([1, H], F32R, tag="ones16", name="ones16")
        nc.sync.dma_start(ones16[:], dt["ones16r"][:].bitcast(F32R))
        epsr = const.tile([1, CH], F32R, tag="epsr", name="epsr")
        nc.sync.dma_start(epsr[:], dt["epsrow"][:].bitcast(F32R))
        brep_o = const.tile([P, D], F32, tag="bor", name="bor")

        # v/q/o weights are not needed until t~100us+: queue their DMAs and
        # trickle them from inside the 1a loop so they don't steal DMA
        # bandwidth from the k weights + xk stream that gate early PE work
        deferred_dmas = []
        wv_sb, vb_sb = {}, {}
        for w in ("1", "2"):
            for lvl in "hl":
                t = wvp.tile([P, 2 * NCD, 512], FP8, tag=f"wv{w}{lvl}",
                             name=f"wv{w}{lvl}")
                deferred_dmas.append((t[:], dt[f"wv{w}8{lvl}"][:]))
                wv_sb[w, lvl] = t
            vb_sb[w] = wvp.tile([P, 2, D], FP8, tag=f"vb{w}", name=f"vb{w}")
            deferred_dmas.append((vb_sb[w][:], dt[f"vb{w}8"][:]))
        wq_sb = {}
        for w in ("1", "2"):
            for lvl in "hl":
                t = wqp.tile([P, NCD, D], FP8, tag=f"wq{w}{lvl}",
                             name=f"wq{w}{lvl}")
                deferred_dmas.append((t[:], dt[f"wq{w}8{lvl}"][:]))
                wq_sb[w, lvl] = t
        wo_sb = {}
        for lvl in "hl":
            t = wqp.tile([P, 2 * NCD, 512], FP8, tag=f"wo{lvl}",
                         name=f"wo{lvl}")
            deferred_dmas.append((t[:], dt[f"wo8{lvl}"][:]))
            wo_sb[lvl] = t
        deferred_dmas.append((brep_o[:], dt["bor"][:]))
        if not has1:
            for dst, src in deferred_dmas:
                nc.gpsimd.dma_start(dst, src)
            deferred_dmas = []

        def glu_pair(p1, p2, x_h, x_l, w_sb, b_sb, n):
            """Two 13-matmul DoubleRow groups, stream-major: the wh-only
            streams (xh+xl) run first for both groups so the wl weight
            tiles can arrive late in the DMA order; the wl streams share
            each xh stationary chunk across p1/p2. p1 closes before p2 so
            its sigmoid overlaps p2's tail."""
            for w, psum in (("1", p1), ("2", p2)):
                for xs in (x_h, x_l):
                    for c in range(4):
                        cs = slice(n * NCD + 2 * c, n * NCD + 2 * c + 2)
                        mm(psum[:], xs[:, 2 * c:2 * c + 2, :],
                           w_sb[w, "h"][:, cs, :],
                           start=(xs is x_h and c == 0), stop=False,
                           perf_mode=DR)
            for c in range(4):
                stat = x_h[:, 2 * c:2 * c + 2, :]
                cs = slice(n * NCD + 2 * c, n * NCD + 2 * c + 2)
                for w, psum in (("1", p1), ("2", p2)):
                    mm(psum[:], stat, w_sb[w, "l"][:, cs, :],
                       start=False, stop=False, perf_mode=DR)
            for w, psum in (("1", p1), ("2", p2)):
                mm(psum[:], onesb[:], b_sb[w][:, :, n * 512:(n + 1) * 512],
                   start=False, stop=True, perf_mode=DR)

        # ================= stage 1a: k projection -> phi_k, ksum ============
        st_ks = ExitStack()  # ksum psum outlives the 1a pools (DMAd at 1b end)
        pksp = st_ks.enter_context(tc.tile_pool(name="pks", bufs=1, space="PSUM"))
        psum_ks = pksp.tile([P, NCD], F32, tag="ks", name="ks") if has1 else None
        with ExitStack() as st1a:
            t1a = st1a.enter_context(tc.tile_pool(name="t1a", bufs=2))
            pk1p = st1a.enter_context(tc.tile_pool(name="pk1", bufs=3, space="PSUM"))
            pk2p = st1a.enter_context(tc.tile_pool(name="pk2", bufs=3, space="PSUM"))

            kq = []
            ks_pending = []

            def ksum_tail(m):
                # moving = mask column: psum_ks[p, j] += Sum_s mask_s *
                # phi_k[s, 128j+p] — 1-row matmuls, ~free on the PE
                for j in range(NCD):
                    mm(psum_ks[:, j:j + 1], phi_k[m][:, j * P:(j + 1) * P],
                       maskc[:, m:m + 1],
                       start=(m == 0), stop=(m == NM - 1),
                       skip_group_check=not (m == 0 or m == NM - 1))

            def flush_exp(upto, gate):
                # Exp batches flush with a 2-tile lag AND gated on one Pool
                # memset: every texp becomes ready at the same instant, so the
                # batch pops contiguously. Without the gate, ready texps leak
                # out one-by-one whenever ACT idles between silus and each
                # leak is a 1283ns table switch.
                batch = [t for t in kq if t[0] <= upto]
                if not batch:
                    return
                bz = t1a.tile([P, 1], F32, tag="bz", name="bz", bufs=2)
                nc.gpsimd.tensor_scalar_mul(bz[:], gate[:, 0:1], 0.0)
                for bm, bn, kg, tmin in batch:
                    ns = slice(bn * 512, (bn + 1) * 512)
                    texp = t1a.tile([P, 512], FP16, tag="texp", name="texp",
                                    bufs=6)
                    nc.scalar.activation(texp[:], tmin[:], ACTF.Exp,
                                         bias=bz[:])
                    # phi = elu+1 = max(kg+1, exp(min(kg,0))): e^x>=1+x
                    nc.vector.scalar_tensor_tensor(phi_k[bm][:, ns], kg[:],
                                                   1.0, texp[:],
                                                   ALU.add, ALU.max)
                    if bn == 1:  # once per tile, not per half
                        ks_pending.append(bm)
                kq[:] = [t for t in kq if t[0] > upto]

            for m in range(NM if has1 else 0):
                if m + 2 < NM:
                    issue_xk(m + 2)
                xk_h, xk_l = xk_tiles.pop(m)
                if m >= 5:
                    for dst, src in deferred_dmas[2 * (m - 5):2 * (m - 4)]:
                        nc.gpsimd.dma_start(dst, src)
                for n in range(2):
                    p1 = pk1p.tile([P, 512], F32, tag="pk1", name="pk1")
                    p2 = pk2p.tile([P, 512], F32, tag="pk2", name="pk2")
                    glu_pair(p1, p2, xk_h, xk_l, wk_sb, kb_sb, n)
                    if n == 0 and m % 2 == 1 and m >= 3:
                        flush_exp(m - 2, tmin_prev)
                    # ksum a tile late (phi via DVE): not-ready matmuls would
                    # clog the 4-deep PE wait queue and starve the engine
                    while ks_pending and ks_pending[0] <= m - 2:
                        ksum_tail(ks_pending.pop(0))
                    g1 = t1a.tile([P, 512], FP16, tag="g1", name="g1")
                    nc.scalar.activation(g1[:], p1[:], ACTF.Silu,
                                         scale=1.0 / WS)
                    kg = t1a.tile([P, 512], FP16, tag="kg", name="kg", bufs=7)
                    nc.vector.scalar_tensor_tensor(kg[:], p2[:], 1.0 / WS, g1[:],
                                                   ALU.mult, ALU.mult)
                    tmin = t1a.tile([P, 512], FP16, tag="tmin", name="tmin",
                                    bufs=7)
                    nc.gpsimd.tensor_scalar_min(tmin[:], kg[:], 0.0)
                    kq.append((m, n, kg, tmin))
                    tmin_prev = tmin
            flush_exp(NM - 1, kq[-1][3])
        st_xk.close()  # frees x_k tiles
        st_k.close()   # frees k weights
        if has2:
            issue_xq(0)
            issue_xq(1)

        # ============== stage 1b: v projection + kv accumulation ============
        with ExitStack() as st1b:
            xvp = st1b.enter_context(tc.tile_pool(name="xv", bufs=6))
            t1b = st1b.enter_context(tc.tile_pool(name="t1b", bufs=3))
            vgp = st1b.enter_context(tc.tile_pool(name="vgp", bufs=4))
            pv1p = st1b.enter_context(tc.tile_pool(name="pv1", bufs=2, space="PSUM"))
            pv2p = st1b.enter_context(tc.tile_pool(name="pv2", bufs=2, space="PSUM"))
            pkvp = st1b.enter_context(tc.tile_pool(name="pkv", bufs=1, space="PSUM"))
            if has1:
                # psum_kv[g] holds heads with h%2==g at plane h//2
                psum_kv = [pkvp.tile([64, NCD, DK], F32, tag=f"pkv{i}",
                                     name=f"pkv{i}") for i in range(2)]

            def kv_tail(m, vg_m):
                # one global accumulation group per bank: start only on the very
                # first matmul (has_written is per element)
                for h in range(H):
                    hs = slice(h * DK, (h + 1) * DK)
                    first = (m == 0 and h < 2)
                    last = (m == NM - 1 and h >= H - 2)
                    nc.tensor.matmul(
                        psum_kv[h % 2][0:64, h // 2, :],
                        phi_k[m][:, hs], vg_m[:, hs],
                        start=first, stop=last,
                        skip_group_check=not (first or last))

            vg_hist = []
            for m in range(NM if has1 else 0):
                xv_h = xvp.tile([P, NCD, P], FP8, tag="xvh", name="xvh")
                xv_l = xvp.tile([P, NCD, P], FP8, tag="xvl", name="xvl")
                nc.sync.dma_start(xv_h[:], dt["xv8h"][:, m * D:(m + 1) * D])
                nc.sync.dma_start(xv_l[:], dt["xv8l"][:, m * D:(m + 1) * D])
                vg = vgp.tile([P, D], FP16, tag="vg", name="vg")
                for n in range(2):
                    ns = slice(n * 512, (n + 1) * 512)
                    p1 = pv1p.tile([P, 512], F32, tag="pv1", name="pv1")
                    p2 = pv2p.tile([P, 512], F32, tag="pv2", name="pv2")
                    glu_pair(p1, p2, xv_h, xv_l, wv_sb, vb_sb, n)
                    g1 = t1b.tile([P, 512], FP16, tag="vg1", name="vg1")
                    nc.scalar.activation(g1[:], p1[:], ACTF.Silu,
                                         scale=1.0 / WS)
                    # mask/WS ptr: vg = silu(t1) * t2 * mask (mask lives here)
                    nc.vector.scalar_tensor_tensor(vg[:, ns], p2[:],
                                                   maskd[:, m:m + 1],
                                                   g1[:], ALU.mult, ALU.mult)
                vg_hist.append(vg)
                if m == 1:
                    for bm in ks_pending:
                        ksum_tail(bm)
                    ks_pending.clear()
                if m >= 2:
                    kv_tail(m - 2, vg_hist[m - 2])
            if has1:
                kv_tail(NM - 2, vg_hist[NM - 2])
                kv_tail(NM - 1, vg_hist[NM - 1])
                # stage psum -> sbuf block-diag (pre-zeroed), then dma out
                nc.scalar.activation(kvstk[0:64, 0:16:2, :], psum_kv[0][:],
                                     ACTF.Copy)
                nc.vector.tensor_copy(kvstk[64:128, 1:16:2, :], psum_kv[1][:])
                nc.vector.tensor_copy(kvstb[0:64, 0:128:18], psum_ks[0:64, :])
                nc.vector.tensor_copy(kvstb[64:128, 1:128:18],
                                      psum_ks[64:128, :])
                nc.sync.dma_start(cc_in[:, 0:1024], kvstk[:])
                nc.gpsimd.dma_start(cc_in[:, 1024:1152], kvstb[:])
        st_ks.close()
        st_v.close()
        st1.close()  # frees phi_k SBUF before stage 2

        # ============ collective: pair AllReduce of kv + ksum ============
        if not has1:
            nc.any.memset(kvstk[:], 1.0)
            nc.any.memset(kvstb[:], 1.0)
            nc.sync.dma_start(cc_in[:, 0:1024], kvstk[:])
            nc.gpsimd.dma_start(cc_in[:, 1024:1152], kvstb[:])
        if single_core:
            nc.sync.dma_start(cc_out[:], cc_in[:])
        else:
            nc.gpsimd.collective_compute(
                "AllReduce", ALU.add, replica_groups=GROUPS,
                ins=[cc_in.opt()], outs=[cc_out.opt()])

        if dbg is not None:
            nc.gpsimd.dma_start(dbg[:], cc_out[:])
        kvp = kvres.tile([P, 1024], F32R, tag="kvp", name="kvp")
        bdt = kvres.tile([P, P], F32R, tag="bdt", name="bdt")
        nc.sync.dma_start(kvp[:], cc_out[:, 0:1024].bitcast(F32R))
        nc.gpsimd.dma_start(bdt[:], cc_out[:, 1024:1152].bitcast(F32R))
        ksum_bd = [bdt[:, H * c:H * (c + 1)] for c in range(NCD)]

        # ============ stage 2: q -> phi_q -> z -> attn -> out ============
        with ExitStack() as st2:
            phiqp = st2.enter_context(tc.tile_pool(name="phiq", bufs=3))
            attnp = st2.enter_context(tc.tile_pool(name="attn", bufs=3))
            t2 = st2.enter_context(tc.tile_pool(name="t2", bufs=4))
            tz = st2.enter_context(tc.tile_pool(name="tz", bufs=2))
            osbp = st2.enter_context(tc.tile_pool(name="osb", bufs=3))
            # 8 PSUM banks: pq (p1|p2 packed) 4, pn 2, po (pz/pr/po) 2
            pqp = st2.enter_context(tc.tile_pool(name="pq", bufs=3, space="PSUM"))
            pnp = st2.enter_context(tc.tile_pool(name="pn", bufs=2, space="PSUM"))
            pop = st2.enter_context(tc.tile_pool(name="po", bufs=3, space="PSUM"))

            def tail_head(phi_q):
                pzt = pop.tile([P, 512], F32, tag="po", name="pzt")
                pz = pzt[0:H, 0:CH]
                mm(pz, ones16[:], epsr[:], start=True, stop=False)  # +EPS
                for c in range(NCD):
                    mm(pz, ksum_bd[c], phi_q[c][:],
                       start=False, stop=(c == NCD - 1))
                r_sb = tz.tile([H, CH], mybir.dt.bfloat16, tag="r_sb",
                               name="r_sb")
                with nc.allow_low_precision(reason="1/z broadcast tolerates bf16"):
                    nc.vector.reciprocal(r_sb[:], pz)
                a_hi = attnp.tile([P, NCD, CH], FP8, tag="ahi", name="ahi")
                a_lo = attnp.tile([P, NCD, CH], FP8, tag="alo", name="alo")
                return r_sb, (a_hi, a_lo)

            def tail_pair(phi_q, r_sb, attn, pair):
                # DVE reads at most one PSUM input: scale phi_q by the
                # broadcast reciprocal first (SBUF x PSUM), then the kv
                # matmuls yield attn directly in PSUM. Both pn halves sit at
                # partition base 0 (base-64 matmul outputs are invalid ISA);
                # the evictions shift head 2p+1 up to partitions 64:128.
                # The [16,128] selector block reads r_sb directly (no copies).
                t = pop.tile([P, 512], F32, tag="po", name="prt")
                pr = t[:, 0:CH]
                mm(pr, blockm[:, pair * P:(pair + 1) * P], r_sb[:, :],
                   start=True, stop=True)
                pqr = tz.tile([P, CH], F32R, tag="pqr", name="pqr", bufs=2)
                nc.vector.tensor_tensor(pqr[:], phi_q[pair][:], pr, ALU.mult)
                # separate PSUM tiles per head: mixing tile-position rows
                # (0 vs 64) inside one PSUM tile crashes the runtime
                pnt = pnp.tile([P, CH], F32, tag="pn", name="pn")
                mm(pnt[:], kvp[:, pair * P:(pair + 1) * P], pqr[:, :],
                   start=True, stop=True)
                a_hi, a_lo = attn
                nc.scalar.activation(a_hi[:, pair, :], pnt[:], ACTF.Copy,
                                     scale=ASC)
                nc.vector.scalar_tensor_tensor(a_lo[:, pair, :], pnt[:], ASC,
                                               a_hi[:, pair, :],
                                               ALU.mult, ALU.subtract)

            def out_mms(po, attn, mt, n, c4):
                # 3 fp8 DoubleRow streams (hi*wh + lo*wh + hi*wl), c-pairs
                a_hi, a_lo = attn
                ts = slice(mt * P, (mt + 1) * P)
                for i, (a, lvl) in enumerate(((a_hi, "h"), (a_lo, "h"),
                                              (a_hi, "l"))):
                    cs = slice(2 * c4, 2 * c4 + 2)
                    ws = slice(n * NCD + 2 * c4, n * NCD + 2 * c4 + 2)
                    mm(po[:], a[:, cs, ts], wo_sb[lvl][:, ws, :],
                       start=(i == 0 and c4 == 0),
                       stop=(i == 2 and c4 == 3), perf_mode=DR)

            def tail_out(ch, attn):
                for mt in range(CH // P):
                    o_sb = osbp.tile([P, D], F32, tag="o_sb", name="o_sb")
                    for n in range(2):
                        ns = slice(n * 512, (n + 1) * 512)
                        po = pop.tile([P, 512], F32, tag="po", name="po")
                        for c4 in range(4):
                            out_mms(po, attn, mt, n, c4)
                        nc.vector.scalar_tensor_tensor(o_sb[:, ns], po[:],
                                                       1.0 / (WS * ASC),
                                                       brep_o[:, ns],
                                                       ALU.mult, ALU.add)
                    row0 = ch * CH + mt * P
                    nc.gpsimd.dma_start(out[row0:row0 + P, :], o_sb[:])

            # sub-stage bisection: stages '2a' = GLU only, '2b' = +tail_head,
            # '2c' = +tail_pair, '2'/'12' = everything
            sub = stages[stages.index("2") + 1:] if "2" in stages else ""
            do_head = sub in ("", "b", "c")
            do_pair = sub in ("", "c")
            do_out = sub == ""
            pending = []
            for ch in range(NCH if has2 else 0):
                if ch + 2 < NCH:
                    issue_xq(ch + 2)
                xq_h, xq_l = xq_tiles.pop(ch)
                # f32r (not fp16): the BIR verifier requires f32r matmul
                # operands to pair with f32r (z/pn read these against
                # f32r ksum_bd/kv_pairs); moving f32r at N=256 is still
                # 1 cycle/row.
                phi_q = [phiqp.tile([P, CH], F32R, tag=f"phiq{mc}",
                                    name=f"phiq{mc}") for mc in range(NCD)]
                # no tail at ch1: the collective gets a full extra chunk of
                # GLU cover; ch2 drains both pending tails
                tails = pending if ch >= 2 else []
                heads = [(p_ch, p_phi, *tail_head(p_phi))
                         for p_ch, p_phi in tails] if do_head else []
                qgs, qtmins = [], []
                for mc in range(NCD):
                    ms = slice(mc * P, (mc + 1) * P)
                    t_q = pqp.tile([P, 512], F32, tag="pq", name="pq")
                    p1, p2 = t_q[:, 0:CH], t_q[:, CH:2 * CH]
                    for psum, w in ((p1, "1"), (p2, "2")):
                        # xh/xl share each wh stationary chunk (fewer Ldweights)
                        for c in range(4):
                            stat = wq_sb[w, "h"][:, 2 * c:2 * c + 2, ms]
                            mm(psum, stat, xq_h[:, 2 * c:2 * c + 2, :],
                               start=(c == 0), stop=False, perf_mode=DR)
                            mm(psum, stat, xq_l[:, 2 * c:2 * c + 2, :],
                               start=False, stop=False, perf_mode=DR)
                        for c in range(4):
                            mm(psum, wq_sb[w, "l"][:, 2 * c:2 * c + 2, ms],
                               xq_h[:, 2 * c:2 * c + 2, :],
                               start=False, stop=(c == 3), perf_mode=DR)
                    # one Silu ACT op; qg carries a WS scale (bq2c = WS*b2 on
                    # host, epsrow = WS*eps) that cancels against 1/z
                    s1 = t2.tile([P, CH], FP16, tag="qs1", name="qs1")
                    nc.scalar.activation(s1[:], p1, ACTF.Silu,
                                         bias=bcol["bq1"][:, mc:mc + 1],
                                         scale=1.0 / WS)
                    qg = t2.tile([P, CH], FP16, tag="qg", name="qg", bufs=NCD)
                    nc.vector.scalar_tensor_tensor(qg[:], p2,
                                                   bcol["bq2"][:, mc:mc + 1],
                                                   s1[:], ALU.add, ALU.mult)
                    tmin = t2.tile([P, CH], FP16, tag="qtmin", name="qtmin",
                                   bufs=NCD)
                    nc.gpsimd.tensor_scalar_min(tmin[:], qg[:], 0.0)
                    qgs.append(qg)
                    qtmins.append(tmin)
                    if do_pair:
                        for p_ch, p_phi, p_rsb, p_attn in heads:
                            tail_pair(p_phi, p_rsb, p_attn, mc)
                texps = []
                # gate the batch on the chunk's last tmin: all texps release
                # together, after every input is ready (bzq = 0*tmin + lnWS)
                bzq = t2.tile([P, 1], F32, tag="bzq", name="bzq", bufs=2)
                nc.gpsimd.tensor_scalar(bzq[:], qtmins[-1][:, 0:1], 0.0, LOGWS,
                                        ALU.mult, ALU.add)
                for mc in range(NCD):  # Exp batch + phi assembly
                    texp = t2.tile([P, CH], FP16, tag="qtexp", name="qtexp",
                                   bufs=NCD)
                    # WS*exp(min(qg,0)) = exp(tmin/WS + ln WS)
                    nc.scalar.activation(texp[:], qtmins[mc][:], ACTF.Exp,
                                         bias=bzq[:], scale=1.0 / WS)
                    texps.append(texp)
                for mc in range(NCD):
                    # phi_q = WS*(elu+1) = max(qg_s + WS, texp_s)
                    nc.vector.scalar_tensor_tensor(phi_q[mc][:], qgs[mc][:], WS,
                                                   texps[mc][:],
                                                   ALU.add, ALU.max)
                if do_out:
                    for p_ch, p_phi, p_rsb, p_attn in heads:
                        tail_out(p_ch, p_attn)
                pending = [t for t in pending if t[0] not in
                           {h[0] for h in heads}]
                pending.append((ch, phi_q))
            if has2 and do_head:
                # drain: out-proj groups (on the free pq banks) trail the
                # attn pairs by one, so PE never waits on an eviction
                # before the next pair's matmuls
                for p_ch, p_phi in pending:
                    p_rsb, p_attn = tail_head(p_phi)
                    if not do_pair:
                        continue
                    units = []
                    if do_out:
                        for mt in range(CH // P):
                            for n in range(2):
                                po = pqp.tile([P, 512], F32, tag="pq",
                                              name="pod")
                                units.append((mt, n, po))

                    def drain_po(c4):
                        for mt, n, po in units:
                            out_mms(po, p_attn, mt, n, c4)

                    for pair in range(NCD):
                        tail_pair(p_phi, p_rsb, p_attn, pair)
                        if pair >= 3 and pair % 2 == 1:
                            drain_po((pair - 3) // 2)
                    drain_po(3)
                    for mt in range(CH // P if do_out else 0):
                        o_sb = osbp.tile([P, D], F32, tag="o_sb", name="o_sb")
                        for mt2, n, po in units:
                            if mt2 == mt:
                                nc.vector.scalar_tensor_tensor(
                                    o_sb[:, n * 512:(n + 1) * 512],
                                    po[:, 0:512], 1.0 / (WS * ASC),
                                    brep_o[:, n * 512:(n + 1) * 512],
                                    ALU.mult, ALU.add)
                        row0 = p_ch * CH + mt * P
                        nc.gpsimd.dma_start(out[row0:row0 + P, :], o_sb[:])


_CACHE = {}


def _get_nc(single_core=False):
    key = bool(single_core)
    if key not in _CACHE:
        _CACHE[key] = build(single_core)
    return _CACHE[key]


def _split8(a):
    hi = a.astype(E4NP)
    lo = (a - hi.astype(np.float32)).astype(E4NP)
    return hi, lo


def _pack_x_tiles(x, tok):
    """x [T, D] f32 -> hi/lo [128, T*8] fp8, cols = blk*(8*tok) + c*tok + t."""
    xf = np.ascontiguousarray(x.T)                       # [D, T]
    a = xf.reshape(NCD, P, T // tok, tok)                # [c, p, blk, t]
    a = a.transpose(1, 2, 0, 3).reshape(P, T * NCD)      # [p, blk*c*t]
    return _split8(np.ascontiguousarray(a))


def _pack_w_moving(w):
    """W [dout, din] -> hi/lo [128, 8192] fp8, cols = n*4096 + c*512 + j."""
    wT = (WS * np.asarray(w, np.float32)).T              # [din, dout]
    a = wT.reshape(NCD, P, 2, 512).transpose(1, 2, 0, 3).reshape(P, 2 * NCD * 512)
    return _split8(np.ascontiguousarray(a))


def _pack_w_stationary(w):
    """W [dout, din] -> hi/lo [128, 8192] fp8, cols = c*1024 + m."""
    wT = (WS * np.asarray(w, np.float32)).T
    a = wT.reshape(NCD, P, D).transpose(1, 0, 2).reshape(P, NCD * D)
    return _split8(np.ascontiguousarray(a))


def _pack_bias(b):
    """b [D] -> [128, 2048] fp8: row0 = [16b_hi | 16b_lo], rest zero."""
    bh, bl = _split8(WS * np.asarray(b, np.float32))
    a = np.zeros((P, 2 * D), E4NP)
    a[0, :D] = bh
    a[0, D:] = bl
    return a


def make_in_maps(inputs):
    f = np.float32
    q = np.asarray(inputs["query"], f).reshape(B * S, D)
    k = np.asarray(inputs["key"], f).reshape(B * S, D)
    v = np.asarray(inputs["value"], f).reshape(B * S, D)
    mask = np.asarray(inputs["mask"], f).reshape(B * S)

    onesb = np.zeros((P, 2 * P), E4NP)
    onesb[0, :] = 1.0
    blockmask = np.zeros((H, NCD * P), ml_dtypes.bfloat16)
    for pr_ in range(NCD):
        blockmask[2 * pr_, pr_ * P:pr_ * P + 64] = 1.0
        blockmask[2 * pr_ + 1, pr_ * P + 64:(pr_ + 1) * P] = 1.0

    shared = {
        "onesb8": onesb,
        "bq1c": np.ascontiguousarray(np.asarray(inputs["q_b1"], f).reshape(NCD, P).T),
        "bq2c": np.ascontiguousarray(WS * np.asarray(inputs["q_b2"], f)
                                     .reshape(NCD, P).T),
        "bor": np.tile(np.asarray(inputs["out_b"], f)[None, :], (P, 1)),
        "blockmask": blockmask,
        "ones16r": np.ones((1, H), f),
        "epsrow": np.full((1, CH), WS * EPS, f),
    }
    woh, wol = _pack_w_moving(inputs["out_w"])
    shared["wo8h"] = woh
    shared["wo8l"] = wol
    for proj, w1k, b1k, w2k, b2k in (("k", "k_w1", "k_b1", "k_w2", "k_b2"),
                                     ("v", "v_w1", "v_b1", "v_w2", "v_b2")):
        for w, wk, bk in (("1", w1k, b1k), ("2", w2k, b2k)):
            hi, lo = _pack_w_moving(inputs[wk])
            shared[f"w{proj}{w}8h"] = hi
            shared[f"w{proj}{w}8l"] = lo
            shared[f"{proj}b{w}8"] = _pack_bias(inputs[bk])
    for w, wk in (("1", "q_w1"), ("2", "q_w2")):
        hi, lo = _pack_w_stationary(inputs[wk])
        shared[f"wq{w}8h"] = hi
        shared[f"wq{w}8l"] = lo

    in_maps = []
    for c in range(NCORES):
        sl = slice(c * T, (c + 1) * T)
        m = dict(shared)
        for nm, x in (("xq", q), ("xk", k), ("xv", v)):
            tok = CH if nm == "xq" else P
            hi, lo = _pack_x_tiles(x[sl], tok)
            m[f"{nm}8h"] = hi
            m[f"{nm}8l"] = lo
        mcol = np.ascontiguousarray(mask[sl].reshape(NM, P).T)
        m["maskp16"] = mcol.astype(np.float16)
        m["maskd16"] = np.ascontiguousarray(mcol / WS)
        in_maps.append(m)
    return in_maps


def kernel(**inputs):
    nc = _get_nc(False)
    in_maps = make_in_maps(inputs)
    res = run_bass_kernel_spmd(nc, in_maps, list(range(NCORES))).results
    outc = np.concatenate([res[c]["out"] for c in range(NCORES)], axis=0)
    return outc.reshape(B, S, D)

